# revision 1
# baseline (speedup 1.0000x reference)
"""DiffAttention Trainium2 Bass kernel (8-core head-parallel SPMD), v3.

Contract: kernel(**inputs) takes the FULL inputs from setup_inputs() and
returns the FULL (B, S, DIM) output. Internally it shards the 16 heads
across 8 NeuronCores (2 heads/core); each core is fully independent (the
reference's transpose-then-reshape makes each head own a contiguous block
of 256 output rows, so no collectives are needed).

v3 design (vs the v2 baseline):
  - all large operands pre-transposed and bf16-cast on the HOST: x^T,
    wq^T/wk^T/wv^T (rope-row-permuted), wo^T (with subln*(1-lambda_init)
    folded in).  No PE transposes of x or weights on device.
  - everything SBUF-resident per batch: no DRAM spill of Q/K/V.
  - attention accumulated TOKEN-major (queries on partitions) with a ones
    column appended to V, so the two softmax denominators fall out of the
    same matmuls that compute attn@V (no separate PE row-sum matmuls).
  - two-stream score matmuls write one [128,1024] PSUM pair tile; a single
    fused Act exp covers both streams; combine uses per-partition-scalar
    DVE ops; RMS via one fused tensor_tensor_reduce.
  - PSUM->SBUF evictions on the (otherwise idle) Pool/GpSimd engine.
"""

import numpy as np
import ml_dtypes
from contextlib import ExitStack

import concourse.bass as bass
import concourse.bacc as bacc
import concourse.tile as tile
from concourse import mybir
from concourse.bass_utils import run_bass_kernel_spmd

F32 = mybir.dt.float32
BF16 = mybir.dt.bfloat16
AF = mybir.ActivationFunctionType
OP = mybir.AluOpType
BFNP = ml_dtypes.bfloat16

B, S, DIM = 2, 2048, 2048
NH, HD, HHD = 16, 128, 64
NC = 8
HPC = NH // NC          # 2 heads per core
E = HPC * HD            # 256 projection rows per core
ND = DIM // 128         # 16 d-tiles
NQT = S // 512          # 4 query strips of 512 per batch
LAMBDA_INIT = 0.2
EPS = 1e-5

_CACHE = {}


def _build_program(nrep=1):
    nc = bacc.Bacc("TRN2", target_bir_lowering=False, debug=False, num_devices=NC)

    xT_d = nc.dram_tensor("xT", [B * DIM, S], BF16, kind="ExternalInput").ap()
    wqT_d = nc.dram_tensor("wqT", [DIM, E], BF16, kind="ExternalInput").ap()
    wkT_d = nc.dram_tensor("wkT", [DIM, E], BF16, kind="ExternalInput").ap()
    wvT_d = nc.dram_tensor("wvT", [DIM, E], BF16, kind="ExternalInput").ap()
    woT_d = nc.dram_tensor("woT", [DIM, DIM], BF16, kind="ExternalInput").ap()
    cosq_d = nc.dram_tensor("cosq", [128, S], BF16, kind="ExternalInput").ap()
    sinq_d = nc.dram_tensor("sinq", [128, S], BF16, kind="ExternalInput").ap()
    cosk_d = nc.dram_tensor("cosk", [128, S], BF16, kind="ExternalInput").ap()
    sink_d = nc.dram_tensor("sink", [128, S], BF16, kind="ExternalInput").ap()
    mask_d = nc.dram_tensor("maskT", [128, 128], BF16, kind="ExternalInput").ap()
    ident_d = nc.dram_tensor("ident", [128, 128], BF16, kind="ExternalInput").ap()
    lamvec_d = nc.dram_tensor("lamvec", [128, 1], F32, kind="ExternalInput").ap()
    identf_d = nc.dram_tensor("identf", [128, 128], mybir.dt.float32r,
                              kind="ExternalInput").ap()
    out_d = nc.dram_tensor("out", [B, E, DIM], F32, kind="ExternalOutput").ap()
    scr_d = nc.dram_tensor("swapscr", [128, S], BF16).ap()

    xT_v = xT_d.rearrange("(b d p) t -> b p d t", b=B, d=ND)

    with tile.TileContext(nc) as tc:
        for rep in range(nrep):
            ctx = ExitStack()
            # ---------------- persistent pools ----------------
            consts = ctx.enter_context(tc.tile_pool(name="consts", bufs=1))
            qkp = ctx.enter_context(tc.tile_pool(name="qk", bufs=1))
            v2p = ctx.enter_context(tc.tile_pool(name="v2", bufs=1))
            atokp = ctx.enter_context(tc.tile_pool(name="atok", bufs=1))
            attnFp = ctx.enter_context(tc.tile_pool(name="attnF", bufs=1))
            exp_ = ctx.enter_context(tc.tile_pool(name="ex", bufs=3))
            swapp = ctx.enter_context(tc.tile_pool(name="swap", bufs=2))
            cmbp = ctx.enter_context(tc.tile_pool(name="cmb", bufs=2))

            atok = {}
            attnF = {}
            for b in range(B):
                for h in range(HPC):
                    atok[(b, h)] = atokp.tile(
                        [128, S], BF16, tag=f"atok{b}{h}", name=f"atok{b}{h}")
                    attnF[(b, h)] = attnFp.tile(
                        [128, S], BF16, tag=f"attnF{b}{h}", name=f"attnF{b}{h}")

            # ------- scoped: x + weights + rope tables (freed before wo) -------
            # the batch-0 x^T DMAs gate the first projections: emit them first.
            wx_ctx = ExitStack()
            xtp = wx_ctx.enter_context(tc.tile_pool(name="xt", bufs=1))
            ropec = wx_ctx.enter_context(tc.tile_pool(name="ropec", bufs=1))
            wp = wx_ctx.enter_context(tc.tile_pool(name="w", bufs=1))

            # x^T tiles keyed by token-block so the first projection only
            # waits for one x tile + wq/wk (3MB), not the full batch.
            xtiles = {}
            wsb = {}

            def _load_xt(b, tb):
                # two half-tiles per token block: matmuls on low d-tiles can
                # start while the high half is still in flight
                for hf in range(2):
                    t_ = xtp.tile([128, (ND // 2) * 512], BF16,
                                  tag=f"xt{tb}{hf}", name=f"xt{tb}{hf}")
                    tv = t_[:].rearrange("p (d t) -> p d t", d=ND // 2)
                    for di in range(ND // 2):
                        nc.sync.dma_start(
                            out=tv[:, di, :],
                            in_=xT_v[b, :, hf * 8 + di,
                                     tb * 512:(tb + 1) * 512])
                    xtiles[(tb, hf)] = tv

            def _load_w(nm, d):
                t_ = wp.tile([128, ND * E], BF16, tag=f"w{nm}", name=f"w{nm}")
                tv = t_[:].rearrange("p (d e) -> p d e", d=ND)
                dv = d.rearrange("(d p) e -> p d e", d=ND)
                for dt in range(ND):
                    nc.sync.dma_start(out=tv[:, dt, :], in_=dv[:, dt, :])
                wsb[nm] = tv

            _load_w("q", wqT_d)
            _load_xt(0, 0)
            _load_w("k", wkT_d)
            for tb in range(1, 4):
                _load_xt(0, tb)
            _load_w("v", wvT_d)

            cs = {}
            for nm, d in (("cosq", cosq_d), ("sinq", sinq_d),
                          ("cosk", cosk_d), ("sink", sink_d)):
                t_ = ropec.tile([128, S], BF16, tag=nm, name=nm)
                nc.sync.dma_start(out=t_, in_=d)
                cs[nm] = t_

            mask_t = consts.tile([128, 128], BF16)
            nc.sync.dma_start(out=mask_t, in_=mask_d)
            ident = consts.tile([128, 128], BF16)
            nc.sync.dma_start(out=ident, in_=ident_d)
            lamvec = consts.tile([128, 1], F32)
            nc.sync.dma_start(out=lamvec, in_=lamvec_d)
            ones_tmp = consts.tile([128, 1], F32)
            nc.vector.memset(ones_tmp, 1.0)
            ones_col = consts.tile([128, 1], F32)
            nc.scalar.copy(out=ones_col.bitcast(mybir.dt.float32r),
                           in_=ones_tmp)
            identf = consts.tile([128, 128], F32)
            nc.sync.dma_start(out=identf.bitcast(mybir.dt.float32r),
                              in_=identf_d)

            for b in range(B):
                # -------- x^T tiles for this batch (b=0 issued at startup) ----
                if b > 0:
                    for tb in range(4):
                        _load_xt(b, tb)

                def xt(dt, tb):
                    return xtiles[(tb, dt // 8)][:, dt % 8, :]

                # -------- projections: Q^T, K^T feature-major per head --------
                qk = {}
                for w in ("q", "k"):
                    for h in range(HPC):
                        qk[(w, h)] = qkp.tile(
                            [128, S], BF16, tag=f"{w}r{h}", name=f"{w}r{h}",
                            bufs=2)
                with tc.tile_pool(name="projP", bufs=3, space="PSUM") as projP:
                    for tb in range(4):
                        for w in ("q", "k"):
                            for h in range(HPC):
                                pp = projP.tile([128, 512], F32, tag="pp",
                                                name="pp")
                                for dt in range(ND):
                                    nc.tensor.matmul(
                                        pp, wsb[w][:, dt, h * 128:(h + 1) * 128],
                                        xt(dt, tb),
                                        start=(dt == 0), stop=(dt == ND - 1))
                                nc.scalar.copy(
                                    out=qk[(w, h)][:, tb * 512:(tb + 1) * 512],
                                    in_=pp)
                    # rope (in-place; swapped copy via SBUF->SBUF DMA)
                    for w in ("q", "k"):
                        for h in range(HPC):
                            t_ = qk[(w, h)]
                            sw = swapp.tile([128, S], BF16, tag="sw", name="sw")
                            nc.sync.dma_start(out=scr_d, in_=t_)
                            for blk in range(4):
                                sb = (blk ^ 1) * 32
                                nc.sync.dma_start(
                                    out=sw[blk * 32:(blk + 1) * 32, :],
                                    in_=scr_d[sb:sb + 32, :])
                            nc.vector.tensor_mul(sw, sw, cs[f"sin{w}"])
                            nc.vector.tensor_mul(t_, t_, cs[f"cos{w}"])
                            nc.vector.tensor_add(t_, t_, sw)

                    # -------- V token-major, both heads, with ones column ----
                    v2 = v2p.tile([128, HPC * (S // 128) * 129], BF16,
                                  tag="v2", name="v2", bufs=2)
                    v2v = v2[:].rearrange("p (h t u) -> p h t u", h=HPC,
                                          t=S // 128)
                    nc.vector.memset(v2, 1.0)
                    for tt in range(S // 128):
                        pv = projP.tile([128, E], F32, tag="pv", name="pv")
                        for dt in range(ND):
                            nc.tensor.matmul(
                                pv,
                                xt(dt, tt // 4)[:, (tt % 4) * 128:
                                                (tt % 4) * 128 + 128],
                                wsb["v"][:, dt, :],
                                start=(dt == 0), stop=(dt == ND - 1))
                        nc.scalar.copy(
                            out=v2v[:, :, tt, 0:128],
                            in_=pv[:].rearrange("p (h u) -> p h u", h=HPC))

                # ---------------- attention per head ----------------
                for h in range(HPC):
                    qr, kr = qk[("q", h)], qk[("k", h)]
                    at_t = atok[(b, h)]
                    with tc.tile_pool(name="spP", bufs=2, space="PSUM") as spP, \
                         tc.tile_pool(name="auP", bufs=1, space="PSUM") as auP:
                        for qt in range(S // 256):
                            i0 = qt * 256
                            njt = 2 * qt + 2
                            au = {(qs, s_): auP.tile(
                                      [128, 512], F32, tag=f"au{qs}{s_}",
                                      name=f"au{qs}{s_}")
                                  for qs in range(2) for s_ in range(2)}
                            for jt in range(njt):
                                r = jt - 2 * qt
                                c0 = max(r, 0) * 128
                                sp = spP.tile([128, 1024], F32, tag="sp",
                                              name="sp")
                                spv = sp[:].rearrange("p (s u) -> p s u", s=2)
                                for s_ in range(2):
                                    e0 = s_ * 64
                                    nc.tensor.matmul(
                                        spv[:, s_, c0:256],
                                        kr[e0:e0 + 64, jt * 128:(jt + 1) * 128],
                                        qr[e0:e0 + 64, i0 + c0:i0 + 256],
                                        start=True, stop=True,
                                        skip_group_check=True)
                                ex = exp_.tile([128, 512], BF16, tag="ex",
                                               name="ex")
                                exv = ex[:].rearrange("p (s u) -> p s u", s=2)
                                nc.scalar.activation(
                                    out=exv[:, :, c0:256],
                                    in_=spv[:, :, c0:256], func=AF.Exp)
                                if r >= 0:
                                    for s_ in range(2):
                                        nc.vector.tensor_mul(
                                            exv[:, s_, c0:c0 + 128],
                                            exv[:, s_, c0:c0 + 128], mask_t)
                                for qs in range(max(r, 0), 2):
                                    stop = (jt == 2 * qt + qs)
                                    for s_ in range(2):
                                        nc.tensor.matmul(
                                            au[(qs, s_)][:, 0:129],
                                            exv[:, s_, qs * 128:(qs + 1) * 128],
                                            v2v[:, h, jt, :],
                                            start=(jt == 0), stop=stop,
                                            skip_group_check=True)
                            for qs in range(2):
                                # RMSNorm is per-token scale-invariant, so
                                # at = au0 - (lam*L0/L1)*au1 suffices: the
                                # 1/L0 factor cancels in the later norm.
                                g = qt * 2 + qs
                                l1r = cmbp.tile([128, 1], F32, tag="l1r",
                                                name="l1r")
                                nc.vector.reciprocal(
                                    l1r, au[(qs, 1)][:, 128:129])
                                rho = cmbp.tile([128, 1], F32, tag="rho",
                                                name="rho")
                                nc.vector.tensor_tensor(
                                    out=rho, in0=au[(qs, 0)][:, 128:129],
                                    in1=l1r, op=OP.mult)
                                nc.vector.tensor_mul(rho, rho, lamvec)
                                t2 = cmbp.tile([128, 128], F32, tag="t2",
                                               name="t2")
                                nc.scalar.activation(
                                    out=t2, in_=au[(qs, 1)][:, 0:128],
                                    func=AF.Copy, scale=rho[:])
                                at = at_t[:, g * 128:(g + 1) * 128]
                                nc.vector.tensor_sub(
                                    at, au[(qs, 0)][:, 0:128], t2)

            wx_ctx.close()

            # ---------------- norm + transpose + output projection ----------
            # the full wo^T sits in the space freed by x^T/weights; its DMA
            # overlaps the tail of attention on batch 1.
            with tc.tile_pool(name="wos", bufs=1) as wosp, \
                 tc.tile_pool(name="nrm", bufs=1) as nrm, \
                 tc.tile_pool(name="ost", bufs=4) as ostp, \
                 tc.tile_pool(name="tpP", bufs=2, space="PSUM") as tpP, \
                 tc.tile_pool(name="poP", bufs=4, space="PSUM") as poP:
                woT_v = woT_d.rearrange("(j p) m -> p j m", j=16)
                wos = wosp.tile([128, 16 * DIM], BF16, tag="wos", name="wos")
                wosv = wos[:].rearrange("p (j m) -> p j m", j=16)
                for jj in range(16):
                    nc.sync.dma_start(out=wosv[:, jj, :], in_=woT_v[:, jj, :])
                for b in range(B):
                    for h in range(HPC):
                        at_t, aF = atok[(b, h)], attnF[(b, h)]
                        aFv = aF[:].rearrange("p (j sg) -> p j sg", j=16)
                        for g in range(S // 128):
                            tnf = nrm.tile([128, 128], F32, tag="tnf",
                                           name="tnf")
                            nc.vector.tensor_copy(
                                out=tnf.bitcast(mybir.dt.float32r),
                                in_=at_t[:, g * 128:(g + 1) * 128])
                            tp = tpP.tile([128, 128], F32, tag="tp", name="tp")
                            nc.tensor.matmul(
                                tp.bitcast(mybir.dt.float32r),
                                tnf.bitcast(mybir.dt.float32r),
                                identf.bitcast(mybir.dt.float32r),
                                is_transpose=True, skip_group_check=True)
                            # scatter-scramble: col s=16*sg+j -> [j, 8g+sg]
                            nc.vector.tensor_copy(
                                out=aFv[:, :, 8 * g:8 * (g + 1)],
                                in_=tp[:].rearrange("p (sg j) -> p j sg", j=16))
                        # RMS over features (partition dim) via ones-matmul,
                        # exactly the v2 baseline's deferred-norm pattern; the
                        # per-column order is scrambled but RMS is per-column.
                        sq = nrm.tile([128, S], F32, tag="sq", name="sq")
                        nc.vector.tensor_mul(sq.bitcast(mybir.dt.float32r),
                                             aF, aF)
                        msb_t = nrm.tile([1, S], F32, tag="msbt", name="msbt")
                        for mq in range(4):
                            msp = tpP.tile([1, 512], F32, tag="msp",
                                           name="msp")
                            nc.tensor.matmul(
                                msp,
                                ones_col.bitcast(mybir.dt.float32r),
                                sq[:, mq * 512:(mq + 1) * 512].bitcast(
                                    mybir.dt.float32r),
                                start=True, stop=True, skip_group_check=True)
                            nc.vector.tensor_scalar(
                                msb_t[:, mq * 512:(mq + 1) * 512], msp,
                                1.0 / 128.0, EPS, OP.mult, OP.add)
                        rinv = nrm.tile([1, S], F32, tag="rinv", name="rinv")
                        nc.vector.reciprocal(rinv, msb_t)
                        rs = nrm.tile([1, S], F32, tag="rs", name="rs")
                        nc.scalar.activation(out=rs, in_=rinv, func=AF.Sqrt)
                        brs = nrm.tile([128, S], F32, tag="brs", name="brs")
                        nc.gpsimd.partition_broadcast(brs, rs)
                        nc.vector.tensor_mul(aF, aF, brs)
                for b in range(B):
                    for h in range(HPC):
                        aF = attnF[(b, h)]
                        for mb in range(4):
                            po = poP.tile([128, 512], F32, tag="po", name="po")
                            for jj in range(16):
                                nc.tensor.matmul(
                                    po, aF[:, jj * 128:(jj + 1) * 128],
                                    wosv[:, jj, mb * 512:(mb + 1) * 512],
                                    start=(jj == 0), stop=(jj == 15))
                            ost = ostp.tile([128, 512], F32, tag="ost",
                                            name="ost")
                            nc.scalar.copy(out=ost, in_=po)
                            nc.sync.dma_start(
                                out=out_d[b, h * 128:(h + 1) * 128,
                                          mb * 512:(mb + 1) * 512],
                                in_=ost)

            ctx.close()

    nc.compile()
    return nc


def get_program(nrep=1):
    key = f"nc{nrep}"
    if key not in _CACHE:
        _CACHE[key] = _build_program(nrep)
    return _CACHE[key]


def _prep_in_maps(inputs):
    inp = {k: np.ascontiguousarray(np.asarray(v, dtype=np.float32))
           for k, v in inputs.items()}
    perm = np.concatenate([
        np.arange(0, 64, 2), np.arange(1, 64, 2),
        np.arange(64, 128, 2), np.arange(65, 128, 2)])
    wq_p = inp["wq"].reshape(NH, HD, DIM)[:, perm, :].reshape(NH * HD, DIM)
    wk_p = inp["wk"].reshape(NH, HD, DIM)[:, perm, :].reshape(NH * HD, DIM)

    fc = inp["freq_cis"]
    cosP = fc[:, :, 0, 0].T.astype(np.float32)
    sinP = fc[:, :, 1, 0].T.astype(np.float32)
    COS = np.concatenate([cosP[0:32], cosP[0:32], cosP[32:64], cosP[32:64]], 0)
    SIN = np.concatenate([-sinP[0:32], sinP[0:32], -sinP[32:64], sinP[32:64]], 0)

    maskT = (np.arange(128)[:, None] <= np.arange(128)[None, :])
    ident = np.eye(128, dtype=np.float32)

    lam1 = np.exp(np.sum(inp["lambda_q1"] * inp["lambda_k1"], dtype=np.float32))
    lam2 = np.exp(np.sum(inp["lambda_q2"] * inp["lambda_k2"], dtype=np.float32))
    lam = lam1 - lam2 + LAMBDA_INIT
    lamvec = np.full((128, 1), lam, np.float32)

    # wo with subln * (1 - lambda_init) folded per scrambled column, transposed
    subw_full = np.tile(inp["subln_w"] * (1.0 - LAMBDA_INIT), NH)  # [2048]
    woT2 = (inp["wo"] * subw_full[None, :]).T  # [k, m]

    xT = np.ascontiguousarray(
        inp["x"].transpose(0, 2, 1)).reshape(B * DIM, S)

    common = {
        "xT": xT.astype(BFNP),
        "woT": np.ascontiguousarray(woT2).astype(BFNP),
        "cosq": np.ascontiguousarray(COS * 0.125).astype(BFNP),
        "sinq": np.ascontiguousarray(SIN * 0.125).astype(BFNP),
        "cosk": np.ascontiguousarray(COS).astype(BFNP),
        "sink": np.ascontiguousarray(SIN).astype(BFNP),
        "maskT": maskT.astype(BFNP),
        "ident": ident.astype(BFNP),
        "lamvec": lamvec,
        "identf": ident,
    }
    in_maps = []
    for c in range(NC):
        m = dict(common)
        m["wqT"] = np.ascontiguousarray(wq_p[c * E:(c + 1) * E].T).astype(BFNP)
        m["wkT"] = np.ascontiguousarray(wk_p[c * E:(c + 1) * E].T).astype(BFNP)
        m["wvT"] = np.ascontiguousarray(inp["wv"][c * E:(c + 1) * E].T).astype(BFNP)
        in_maps.append(m)
    return in_maps


def run(inputs, trace=False, **kw):
    nc = get_program()
    in_maps = _prep_in_maps(inputs)
    res = run_bass_kernel_spmd(nc, in_maps, list(range(NC)), trace=trace, **kw)
    out = np.empty((B, S, DIM), np.float32)
    for c in range(NC):
        out[:, c * E:(c + 1) * E, :] = np.asarray(
            res.results[c]["out"]).astype(np.float32)
    return out, res


def kernel(**inputs):
    out, _ = run(inputs)
    return out


# ---------------------------------------------------------------------------
# benchmarking helpers (wall-clock with device-resident inputs, null-calibrated)
# ---------------------------------------------------------------------------

def _make_sharded_callable(nc, in_maps, n_cores):
    import jax
    from jax.experimental.shard_map import shard_map
    from jax.sharding import Mesh, PartitionSpec, NamedSharding
    from concourse import bass2jax

    bass2jax.install_neuronx_cc_hook()
    partition_name = nc.partition_id_tensor.name if nc.partition_id_tensor else None
    in_names, out_names, out_avals, zero_outs = [], [], [], []
    for alloc in nc.m.functions[0].allocations:
        if not isinstance(alloc, mybir.MemoryLocationSet):
            continue
        name = alloc.memorylocations[0].name
        if alloc.kind == "ExternalInput":
            if name != partition_name:
                in_names.append(name)
        elif alloc.kind == "ExternalOutput":
            out_names.append(name)
            shape = tuple(alloc.tensor_shape)
            dtype = mybir.dt.np(alloc.dtype)
            out_avals.append(jax.core.ShapedArray(shape, dtype))
            zero_outs.append(np.zeros(shape, dtype))
    n_params = len(in_names)
    all_in = list(in_names) + list(out_names)
    if partition_name is not None:
        all_in.append(partition_name)

    def _body(*args):
        operands = list(args)
        if partition_name is not None:
            operands.append(bass2jax.partition_id_tensor())
        outs = bass2jax._bass_exec_p.bind(
            *operands,
            out_avals=tuple(out_avals),
            in_names=tuple(all_in),
            out_names=tuple(out_names),
            lowering_input_output_aliases=(),
            sim_require_finite=True,
            sim_require_nnan=True,
            nc=nc,
        )
        return tuple(outs)

    devices = jax.devices()[:n_cores]
    mesh = Mesh(np.asarray(devices), ("core",))
    in_specs = (PartitionSpec("core"),) * (n_params + len(out_names))
    out_specs = (PartitionSpec("core"),) * len(out_names)
    fn = jax.jit(shard_map(_body, mesh=mesh, in_specs=in_specs,
                           out_specs=out_specs, check_rep=False),
                 keep_unused=True)
    sh = NamedSharding(mesh, PartitionSpec("core"))
    per_core = [[np.asarray(m[n]) for n in in_names] for m in in_maps]
    args = [np.concatenate([per_core[c][i] for c in range(n_cores)], axis=0)
            for i in range(n_params)]
    args += [np.zeros((n_cores * z.shape[0], *z.shape[1:]), z.dtype)
             for z in zero_outs]
    dev_args = [jax.device_put(a, sh) for a in args]
    return fn, dev_args


def _time_calls(fn, dev_args, iters=8):
    import time as _t
    import jax
    out = fn(*dev_args)
    jax.block_until_ready(out)
    times = []
    for _ in range(iters):
        t0 = _t.perf_counter()
        out = fn(*dev_args)
        jax.block_until_ready(out)
        times.append(_t.perf_counter() - t0)
    return min(times), times


def bench_rep(inputs, nrep=3, iters=20):
    """(T_nrep - T_1)/(nrep-1) from min wall times; dispatch cancels."""
    in_maps = _prep_in_maps(inputs)
    nc1 = get_program(1)
    fn1, dev1 = _make_sharded_callable(nc1, in_maps, NC)
    t1, t1_all = _time_calls(fn1, dev1, iters)
    ncN = get_program(nrep)
    fnN, devN = _make_sharded_callable(ncN, in_maps, NC)
    tN, tN_all = _time_calls(fnN, devN, iters)
    per = (tN - t1) / (nrep - 1)
    return per, t1, tN, t1_all, tN_all

